# revision 7
# baseline (speedup 1.0000x reference)
"""Trainium2 Bass kernel for MultiHeadAttention (B=4, S=1024, D=1024, H=16).

Sharding: 8 cores = (batch b in 0..3) x (head-group g in 0..1, 8 heads each).
Each core computes, for its (b, g):
  - qhT/khT = (Wq_g/8) @ q[b]^T, Wk_g @ k[b]^T   (head-transposed projections)
  - vh      = v[b] @ Wv_g^T                       (natural layout, + ones column)
  - per head: logitsT[sk,sq] = khT^T-style matmul + adjoinT (mask folded in),
              expT = exp(logitsT)  (written unnormalized to HBM),
              ctx~T/Z via single matmul against ones-augmented vh,
              ctxT = ctx~T * (1/Z) + wv_b
  - out_partial[sq,e] = ctxT_g^T @ dense_w_g^T    (row-parallel dense)
Host: pre-transposes inputs/weights, then normalizes+transposes attention
weights and sum-reduces the two dense partials per batch.
"""

import sys

if "/opt/trn_rl_repo" not in sys.path:
    sys.path.insert(0, "/opt/trn_rl_repo")

import numpy as np

import concourse.bass as bass  # noqa: F401  (registers types)
import concourse.tile as tile
from concourse import bacc, mybir
from concourse.bass_utils import run_bass_kernel_spmd

P = 128
S = 1024
D = 1024
H = 16
HC = 8        # heads per core
DEPTH = 64
W = 512       # local (per-core) head width = HC * DEPTH
F32 = mybir.dt.float32

# Matmul compute dtype: float32 (exact) or float32r (4x faster, reduced
# precision multiplies). Chosen empirically against the fp32 reference.
MM_DT = mybir.dt.float32r


def _mm(ap, mm_dt):
    return ap.bitcast(mm_dt) if mm_dt != F32 else ap


def build_program(mm_dt=MM_DT, es_dt=None):
    if es_dt is None:
        es_dt = mm_dt
    nc = bacc.Bacc("TRN2", target_bir_lowering=False, debug=False, num_devices=8)

    qT = nc.dram_tensor("qT", [D, S], mm_dt, kind="ExternalInput").ap()
    kT = nc.dram_tensor("kT", [D, S], mm_dt, kind="ExternalInput").ap()
    vT = nc.dram_tensor("vT", [D, S], mm_dt, kind="ExternalInput").ap()
    adjT_d = nc.dram_tensor("adjT", [S, S], mm_dt, kind="ExternalInput").ap()
    wqT = nc.dram_tensor("wqT", [D, W], mm_dt, kind="ExternalInput").ap()
    wkT = nc.dram_tensor("wkT", [D, W], mm_dt, kind="ExternalInput").ap()
    wvT = nc.dram_tensor("wvT", [D, W], mm_dt, kind="ExternalInput").ap()
    dwT_d = nc.dram_tensor("dwT", [W, D], mm_dt, kind="ExternalInput").ap()
    qb_d = nc.dram_tensor("qb", [P, 4], F32, kind="ExternalInput").ap()
    kb_d = nc.dram_tensor("kb", [P, 4], F32, kind="ExternalInput").ap()
    vb_d = nc.dram_tensor("vb", [P, 4], F32, kind="ExternalInput").ap()

    attn_un = nc.dram_tensor("attn_un", [HC, S, S], es_dt, kind="ExternalOutput").ap()
    outp = nc.dram_tensor("outp", [S, D], F32, kind="ExternalOutput").ap()

    ADD = mybir.AluOpType.add
    MULT = mybir.AluOpType.mult
    EXP = mybir.ActivationFunctionType.Exp

    with tile.TileContext(nc) as tc:
        with tc.tile_pool(name="const", bufs=1) as const, \
             tc.tile_pool(name="persist", bufs=1) as persist:
            ones = const.tile([P, DEPTH], F32, tag="ones")
            nc.vector.memset(ones[:], 1.0)
            from concourse.masks import make_identity
            ident_f32 = const.tile([P, P], F32, tag="identf")
            make_identity(nc, ident_f32)
            if mm_dt != F32:
                ident = const.tile([P, P], mm_dt, tag="ident")
                nc.vector.tensor_copy(out=ident[:], in_=ident_f32[:])
            else:
                ident = ident_f32
            qb_sb = const.tile([P, 4], F32, tag="qb")
            kb_sb = const.tile([P, 4], F32, tag="kb")
            vb_sb = const.tile([P, 4], F32, tag="vb")
            nc.sync.dma_start(out=qb_sb[:], in_=qb_d[:])
            nc.sync.dma_start(out=kb_sb[:], in_=kb_d[:])
            nc.sync.dma_start(out=vb_sb[:], in_=vb_d[:])

            qhT = [persist.tile([P, S], mm_dt, tag=f"qhT{m}", name=f"qhT{m}") for m in range(4)]
            khT = [persist.tile([P, S], mm_dt, tag=f"khT{m}", name=f"khT{m}") for m in range(4)]
            vh = [persist.tile([P, HC, DEPTH + 1], es_dt, tag=f"vh{m}", name=f"vh{m}")
                  for m in range(8)]
            ctxT = [persist.tile([P, S], mm_dt, tag=f"ctxT{m}", name=f"ctxT{m}") for m in range(4)]
            adjT = [persist.tile([P, S], mm_dt, tag=f"adjT{m}", name=f"adjT{m}") for m in range(8)]
            dwT = [persist.tile([P, S], mm_dt, tag=f"dwT{m}", name=f"dwT{m}") for m in range(4)]

            for m in range(8):
                nc.sync.dma_start(out=adjT[m][:], in_=adjT_d[m * P:(m + 1) * P, :])
            for m in range(4):
                nc.sync.dma_start(out=dwT[m][:], in_=dwT_d[m * P:(m + 1) * P, :])
            for m in range(8):
                nc.vector.tensor_copy(out=vh[m][:, :, DEPTH:DEPTH + 1],
                                      in_=ones[:, 0:HC])

            # ---- Phase A: projections ----
            with tc.tile_pool(name="acts", bufs=3) as actp, \
                 tc.tile_pool(name="wts", bufs=8) as wtp, \
                 tc.tile_pool(name="psA", bufs=1, space="PSUM") as psA:

                def proj_qkT(src, w_src, bias_sb, outT):
                    ps = {}
                    for m in range(4):
                        for n in range(2):
                            ps[(m, n)] = psA.tile([P, 512], F32, tag=f"pj{m * 2 + n}", name=f"pj{m * 2 + n}")
                    for kd in range(8):
                        w_t = wtp.tile([P, W], mm_dt, tag="w")
                        nc.sync.dma_start(out=w_t[:],
                                          in_=w_src[kd * P:(kd + 1) * P, :])
                        a_t = actp.tile([P, S], mm_dt, tag="a")
                        nc.sync.dma_start(out=a_t[:],
                                          in_=src[kd * P:(kd + 1) * P, :])
                        for m in range(4):
                            for n in range(2):
                                nc.tensor.matmul(
                                    ps[(m, n)][:],
                                    w_t[:, m * P:(m + 1) * P],
                                    a_t[:, n * 512:(n + 1) * 512],
                                    start=(kd == 0), stop=(kd == 7))
                    for m in range(4):
                        for n in range(2):
                            nc.vector.tensor_scalar(
                                out=outT[m][:, n * 512:(n + 1) * 512],
                                in0=ps[(m, n)][:],
                                scalar1=bias_sb[:, m:m + 1],
                                scalar2=None, op0=ADD)

                def proj_v():
                    ps = [psA.tile([P, W], F32, tag=f"pj{i}", name=f"pj{i}") for i in range(8)]
                    for kd in range(8):
                        w_t = wtp.tile([P, W], mm_dt, tag="w")
                        nc.sync.dma_start(out=w_t[:],
                                          in_=wvT[kd * P:(kd + 1) * P, :])
                        a_t = actp.tile([P, S], mm_dt, tag="a")
                        nc.sync.dma_start(out=a_t[:],
                                          in_=vT[kd * P:(kd + 1) * P, :])
                        for m in range(8):
                            nc.tensor.matmul(
                                ps[m][:],
                                a_t[:, m * P:(m + 1) * P],
                                w_t[:],
                                start=(kd == 0), stop=(kd == 7))
                    for m in range(8):
                        nc.vector.tensor_copy(
                            out=vh[m][:, :, 0:DEPTH],
                            in_=ps[m][:].rearrange("p (h d) -> p h d", h=HC))

                proj_qkT(qT, wqT, qb_sb, qhT)
                proj_qkT(kT, wkT, kb_sb, khT)
                proj_v()

            # ---- Phase B: attention per head ----
            with tc.tile_pool(name="lsb", bufs=3) as lsp, \
                 tc.tile_pool(name="esb", bufs=3) as esp, \
                 tc.tile_pool(name="zsb", bufs=2) as zsp, \
                 tc.tile_pool(name="plp", bufs=2, space="PSUM") as plp, \
                 tc.tile_pool(name="pcp", bufs=2, space="PSUM") as pcp:
                for h in range(HC):
                    mch = h // 2
                    poff = (h % 2) * DEPTH
                    q_h = qhT[mch][poff:poff + DEPTH, :]
                    k_h = khT[mch][poff:poff + DEPTH, :]
                    pc = pcp.tile([DEPTH + 1, S], F32, tag="pc")
                    for sk in range(8):
                        pl = plp.tile([P, S], F32, tag="pl")
                        for n in range(2):
                            nc.tensor.matmul(
                                pl[:, n * 512:(n + 1) * 512],
                                k_h[:, sk * P:(sk + 1) * P],
                                q_h[:, n * 512:(n + 1) * 512],
                                start=True, stop=False)
                            # accumulate adjoin via identity matmul (PE),
                            # keeping the DVE off the critical path
                            nc.tensor.matmul(
                                pl[:, n * 512:(n + 1) * 512],
                                ident[:],
                                adjT[sk][:, n * 512:(n + 1) * 512],
                                start=False, stop=True)
                        es = esp.tile([P, S], es_dt, tag="es")
                        nc.scalar.activation(es[:], pl[:], EXP)
                        nc.sync.dma_start(
                            out=attn_un[h, sk * P:(sk + 1) * P, :], in_=es[:])
                        for n in range(2):
                            nc.tensor.matmul(
                                pc[:, n * 512:(n + 1) * 512],
                                vh[sk][:, h, :],
                                es[:, n * 512:(n + 1) * 512],
                                start=(sk == 0), stop=(sk == 7))
                    # Z (partition 64 of pc) -> 1/Z -> broadcast across
                    # partitions (GpSimd) -> normalize ctx~T.
                    zz = zsp.tile([1, S], F32, tag="zz")
                    nc.vector.tensor_copy(out=zz[0:1, :],
                                          in_=pc[DEPTH:DEPTH + 1, :])
                    rz = zsp.tile([1, S], F32, tag="rz")
                    nc.vector.reciprocal_approx_fast(out=rz[0:1, :],
                                                     in_=zz[0:1, :])
                    rzb = zsp.tile([DEPTH, S], F32, tag="rzb")
                    nc.gpsimd.partition_broadcast(rzb[:], rz[0:1, :],
                                                  channels=DEPTH)
                    ct = ctxT[mch]
                    nc.vector.tensor_tensor(out=ct[poff:poff + DEPTH, :],
                                            in0=pc[0:DEPTH, :], in1=rzb[:],
                                            op=MULT)
                    nc.vector.tensor_scalar(
                        out=ct[poff:poff + DEPTH, :],
                        in0=ct[poff:poff + DEPTH, :],
                        scalar1=vb_sb[poff:poff + DEPTH, mch:mch + 1],
                        scalar2=None, op0=ADD)

            # ---- Phase C: dense (row-parallel partial) ----
            with tc.tile_pool(name="osb", bufs=3) as osp, \
                 tc.tile_pool(name="psC", bufs=4, space="PSUM") as psC:
                for m in range(8):
                    os_t = osp.tile([P, S], F32, tag="os")
                    for n in range(2):
                        pd = psC.tile([P, 512], F32, tag="pd")
                        for kc in range(4):
                            nc.tensor.matmul(
                                pd[:],
                                ctxT[kc][:, m * P:(m + 1) * P],
                                dwT[kc][:, n * 512:(n + 1) * 512],
                                start=(kc == 0), stop=(kc == 3))
                        nc.vector.tensor_copy(
                            out=os_t[:, n * 512:(n + 1) * 512], in_=pd[:])
                    nc.sync.dma_start(out=outp[m * P:(m + 1) * P, :],
                                      in_=os_t[:])

    nc.compile()
    return nc


_PROGRAM_CACHE = {}


def get_program(mm_dt=MM_DT, es_dt=None):
    key = (str(mm_dt), str(es_dt))
    if key not in _PROGRAM_CACHE:
        _PROGRAM_CACHE[key] = build_program(mm_dt, es_dt)
    return _PROGRAM_CACHE[key]


def make_in_maps(v, k, q, mask, adjoin_matrix,
                 wq_w, wq_b, wk_w, wk_b, wv_w, wv_b, dense_w, dense_b):
    c = np.ascontiguousarray
    f32 = np.float32
    in_maps = []
    per_batch = {}
    for b in range(4):
        per_batch[b] = {
            "qT": c(np.asarray(q[b], f32).T),
            "kT": c(np.asarray(k[b], f32).T),
            "vT": c(np.asarray(v[b], f32).T),
            "adjT": c(np.asarray(adjoin_matrix[b, 0], f32).T)
            + np.float32(-1e9) * np.asarray(mask[b, 0, 0], f32)[:, None],
        }
    for cid in range(8):
        b, g = cid // 2, cid % 2
        gs = slice(g * W, (g + 1) * W)
        m = dict(per_batch[b])
        m["wqT"] = c(np.asarray(wq_w, f32)[gs].T) * f32(0.125)
        m["wkT"] = c(np.asarray(wk_w, f32)[gs].T)
        m["wvT"] = c(np.asarray(wv_w, f32)[gs].T)
        m["dwT"] = c(np.asarray(dense_w, f32)[:, gs].T)
        m["qb"] = c((np.asarray(wq_b, f32)[gs] * f32(0.125)).reshape(4, P).T)
        m["kb"] = c(np.asarray(wk_b, f32)[gs].reshape(4, P).T)
        m["vb"] = c(np.asarray(wv_b, f32)[gs].reshape(4, P).T)
        in_maps.append(m)
    return in_maps


def assemble_outputs(results, dense_b):
    out = np.empty((4, S, D), np.float32)
    attn = np.empty((4, H, S, S), np.float32)
    for cid in range(8):
        b, g = cid // 2, cid % 2
        au = results[cid]["attn_un"]          # [HC, sk, sq]
        z = au.sum(axis=1)                    # [HC, sq]
        attn[b, g * HC:(g + 1) * HC] = (au / z[:, None, :]).transpose(0, 2, 1)
    db = np.asarray(dense_b, np.float32)
    for b in range(4):
        out[b] = results[2 * b]["outp"] + results[2 * b + 1]["outp"] + db
    return out, attn


def run_cores(inputs, mm_dt=MM_DT, es_dt=None, trace=False, **run_kwargs):
    nc = get_program(mm_dt, es_dt)
    in_maps = make_in_maps(**inputs)
    res = run_bass_kernel_spmd(nc, in_maps, core_ids=list(range(8)),
                               trace=trace, **run_kwargs)
    return res


def kernel(**inputs):
    res = run_cores(inputs)
    return assemble_outputs(res.results, inputs["dense_b"])


# revision 8
# speedup vs baseline: 1.1542x; 1.1542x over previous
"""Trainium2 Bass kernel for MultiHeadAttention (B=4, S=1024, D=1024, H=16).

Sharding: 8 cores = (batch b in 0..3) x (head-group g in 0..1, 8 heads each).
Each core computes, for its (b, g):
  - qhT/khT = (Wq_g/8) @ q[b]^T, Wk_g @ k[b]^T   (head-transposed projections)
  - vh      = v[b] @ Wv_g^T                       (natural layout, + ones column)
  - per head: logitsT[sk,sq] = khT^T-style matmul + adjoinT (mask folded in),
              expT = exp(logitsT)  (written unnormalized to HBM),
              ctx~T/Z via single matmul against ones-augmented vh,
              ctxT = ctx~T * (1/Z) + wv_b
  - out_partial[sq,e] = ctxT_g^T @ dense_w_g^T    (row-parallel dense)
Host: pre-transposes inputs/weights, then normalizes+transposes attention
weights and sum-reduces the two dense partials per batch.
"""

import sys

if "/opt/trn_rl_repo" not in sys.path:
    sys.path.insert(0, "/opt/trn_rl_repo")

import numpy as np

import concourse.bass as bass  # noqa: F401  (registers types)
import concourse.tile as tile
from concourse import bacc, mybir
from concourse.bass_utils import run_bass_kernel_spmd

P = 128
S = 1024
D = 1024
H = 16
HC = 8        # heads per core
DEPTH = 64
W = 512       # local (per-core) head width = HC * DEPTH
F32 = mybir.dt.float32

# Matmul compute dtype: float32 (exact) or float32r (4x faster, reduced
# precision multiplies). Chosen empirically against the fp32 reference.
MM_DT = mybir.dt.float32r


def _mm(ap, mm_dt):
    return ap.bitcast(mm_dt) if mm_dt != F32 else ap


def build_program(mm_dt=MM_DT, es_dt=None):
    if es_dt is None:
        es_dt = mm_dt
    nc = bacc.Bacc("TRN2", target_bir_lowering=False, debug=False, num_devices=8)

    qT = nc.dram_tensor("qT", [D, S], mm_dt, kind="ExternalInput").ap()
    kT = nc.dram_tensor("kT", [D, S], mm_dt, kind="ExternalInput").ap()
    vT = nc.dram_tensor("vT", [D, S], mm_dt, kind="ExternalInput").ap()
    adjT_d = nc.dram_tensor("adjT", [S, S], mm_dt, kind="ExternalInput").ap()
    wqT = nc.dram_tensor("wqT", [D, W], mm_dt, kind="ExternalInput").ap()
    wkT = nc.dram_tensor("wkT", [D, W], mm_dt, kind="ExternalInput").ap()
    wvT = nc.dram_tensor("wvT", [D, W], mm_dt, kind="ExternalInput").ap()
    dwT_d = nc.dram_tensor("dwT", [W, D], mm_dt, kind="ExternalInput").ap()
    qb_d = nc.dram_tensor("qb", [P, 4], F32, kind="ExternalInput").ap()
    kb_d = nc.dram_tensor("kb", [P, 4], F32, kind="ExternalInput").ap()
    vb_d = nc.dram_tensor("vb", [P, 4], F32, kind="ExternalInput").ap()

    attn_un = nc.dram_tensor("attn_un", [HC, S, S], es_dt, kind="ExternalOutput").ap()
    outp = nc.dram_tensor("outp", [S, D], F32, kind="ExternalOutput").ap()

    ADD = mybir.AluOpType.add
    MULT = mybir.AluOpType.mult
    EXP = mybir.ActivationFunctionType.Exp

    with tile.TileContext(nc) as tc:
        with tc.tile_pool(name="const", bufs=1) as const, \
             tc.tile_pool(name="persist", bufs=1) as persist:
            ones = const.tile([P, DEPTH], F32, tag="ones")
            nc.vector.memset(ones[:], 1.0)
            from concourse.masks import make_identity
            ident_f32 = const.tile([P, P], F32, tag="identf")
            make_identity(nc, ident_f32)
            if mm_dt != F32:
                ident = const.tile([P, P], mm_dt, tag="ident")
                nc.vector.tensor_copy(out=ident[:], in_=ident_f32[:])
            else:
                ident = ident_f32
            qb_sb = const.tile([P, 4], F32, tag="qb")
            kb_sb = const.tile([P, 4], F32, tag="kb")
            vb_sb = const.tile([P, 4], F32, tag="vb")
            nc.sync.dma_start(out=qb_sb[:], in_=qb_d[:])
            nc.sync.dma_start(out=kb_sb[:], in_=kb_d[:])
            nc.sync.dma_start(out=vb_sb[:], in_=vb_d[:])

            qhT = [persist.tile([P, S], mm_dt, tag=f"qhT{m}", name=f"qhT{m}") for m in range(4)]
            khT = [persist.tile([P, S], mm_dt, tag=f"khT{m}", name=f"khT{m}") for m in range(4)]
            vh = [persist.tile([P, HC, DEPTH + 1], es_dt, tag=f"vh{m}", name=f"vh{m}")
                  for m in range(8)]
            ctxT = [persist.tile([P, S], mm_dt, tag=f"ctxT{m}", name=f"ctxT{m}") for m in range(4)]
            adjT = [persist.tile([P, S], mm_dt, tag=f"adjT{m}", name=f"adjT{m}") for m in range(8)]
            dwT = [persist.tile([P, S], mm_dt, tag=f"dwT{m}", name=f"dwT{m}") for m in range(4)]

            for m in range(8):
                nc.sync.dma_start(out=adjT[m][:], in_=adjT_d[m * P:(m + 1) * P, :])
            for m in range(4):
                nc.sync.dma_start(out=dwT[m][:], in_=dwT_d[m * P:(m + 1) * P, :])
            for m in range(8):
                nc.vector.tensor_copy(out=vh[m][:, :, DEPTH:DEPTH + 1],
                                      in_=ones[:, 0:HC])

            # ---- Phase A: projections ----
            with tc.tile_pool(name="acts", bufs=3) as actp, \
                 tc.tile_pool(name="wts", bufs=8) as wtp, \
                 tc.tile_pool(name="psA", bufs=1, space="PSUM") as psA:

                def proj_qkT(src, w_src, bias_sb, outT):
                    ps = {}
                    for m in range(4):
                        for n in range(2):
                            ps[(m, n)] = psA.tile([P, 512], F32, tag=f"pj{m * 2 + n}", name=f"pj{m * 2 + n}")
                    for kd in range(8):
                        w_t = wtp.tile([P, W], mm_dt, tag="w")
                        nc.sync.dma_start(out=w_t[:],
                                          in_=w_src[kd * P:(kd + 1) * P, :])
                        a_t = actp.tile([P, S], mm_dt, tag="a")
                        nc.sync.dma_start(out=a_t[:],
                                          in_=src[kd * P:(kd + 1) * P, :])
                        for m in range(4):
                            for n in range(2):
                                nc.tensor.matmul(
                                    ps[(m, n)][:],
                                    w_t[:, m * P:(m + 1) * P],
                                    a_t[:, n * 512:(n + 1) * 512],
                                    start=(kd == 0), stop=(kd == 7))
                    for m in range(4):
                        for n in range(2):
                            nc.vector.tensor_scalar(
                                out=outT[m][:, n * 512:(n + 1) * 512],
                                in0=ps[(m, n)][:],
                                scalar1=bias_sb[:, m:m + 1],
                                scalar2=None, op0=ADD)

                def proj_v():
                    ps = [psA.tile([P, W], F32, tag=f"pj{i}", name=f"pj{i}") for i in range(8)]
                    for kd in range(8):
                        w_t = wtp.tile([P, W], mm_dt, tag="w")
                        nc.sync.dma_start(out=w_t[:],
                                          in_=wvT[kd * P:(kd + 1) * P, :])
                        a_t = actp.tile([P, S], mm_dt, tag="a")
                        nc.sync.dma_start(out=a_t[:],
                                          in_=vT[kd * P:(kd + 1) * P, :])
                        for m in range(8):
                            nc.tensor.matmul(
                                ps[m][:],
                                a_t[:, m * P:(m + 1) * P],
                                w_t[:],
                                start=(kd == 0), stop=(kd == 7))
                    for m in range(8):
                        nc.vector.tensor_copy(
                            out=vh[m][:, :, 0:DEPTH],
                            in_=ps[m][:].rearrange("p (h d) -> p h d", h=HC))

                proj_qkT(qT, wqT, qb_sb, qhT)
                proj_qkT(kT, wkT, kb_sb, khT)
                proj_v()

            # ---- Phase B: attention per head ----
            with tc.tile_pool(name="lsb", bufs=3) as lsp, \
                 tc.tile_pool(name="esb", bufs=9) as esp, \
                 tc.tile_pool(name="zsb", bufs=2) as zsp, \
                 tc.tile_pool(name="plp", bufs=2, space="PSUM") as plp, \
                 tc.tile_pool(name="pcp", bufs=2, space="PSUM") as pcp:
                for h in range(HC):
                    mch = h // 2
                    poff = (h % 2) * DEPTH
                    q_h = qhT[mch][poff:poff + DEPTH, :]
                    k_h = khT[mch][poff:poff + DEPTH, :]
                    # Pass 1: logits + adjoin (PE) -> exp (ACT, reads PSUM).
                    # No PE instruction in this pass depends on ACT output,
                    # so the in-order PE queue never stalls on exp.
                    es_tiles = []
                    for sk in range(8):
                        pl = plp.tile([P, S], F32, tag="pl")
                        for n in range(2):
                            nc.tensor.matmul(
                                pl[:, n * 512:(n + 1) * 512],
                                k_h[:, sk * P:(sk + 1) * P],
                                q_h[:, n * 512:(n + 1) * 512],
                                start=True, stop=False)
                            # accumulate adjoin via identity matmul (PE),
                            # keeping the DVE off the critical path
                            nc.tensor.matmul(
                                pl[:, n * 512:(n + 1) * 512],
                                ident[:],
                                adjT[sk][:, n * 512:(n + 1) * 512],
                                start=False, stop=True)
                        es = esp.tile([P, S], es_dt, tag="es", name="es")
                        nc.scalar.activation(es[:], pl[:], EXP)
                        nc.sync.dma_start(
                            out=attn_un[h, sk * P:(sk + 1) * P, :], in_=es[:])
                        es_tiles.append(es)
                    # Pass 2: ctx~T accumulation; es[sk] is ready well before
                    # the PE reaches the corresponding matmul.
                    pc = pcp.tile([DEPTH + 1, S], F32, tag="pc")
                    for sk in range(8):
                        for n in range(2):
                            nc.tensor.matmul(
                                pc[:, n * 512:(n + 1) * 512],
                                vh[sk][:, h, :],
                                es_tiles[sk][:, n * 512:(n + 1) * 512],
                                start=(sk == 0), stop=(sk == 7))
                    # Z (partition 64 of pc) -> 1/Z -> broadcast across
                    # partitions (GpSimd) -> normalize ctx~T.
                    zz = zsp.tile([1, S], F32, tag="zz")
                    nc.vector.tensor_copy(out=zz[0:1, :],
                                          in_=pc[DEPTH:DEPTH + 1, :])
                    rz = zsp.tile([1, S], F32, tag="rz")
                    nc.vector.reciprocal_approx_fast(out=rz[0:1, :],
                                                     in_=zz[0:1, :])
                    rzb = zsp.tile([DEPTH, S], F32, tag="rzb")
                    nc.gpsimd.partition_broadcast(rzb[:], rz[0:1, :],
                                                  channels=DEPTH)
                    ct = ctxT[mch]
                    nc.vector.tensor_tensor(out=ct[poff:poff + DEPTH, :],
                                            in0=pc[0:DEPTH, :], in1=rzb[:],
                                            op=MULT)
                    nc.vector.tensor_scalar(
                        out=ct[poff:poff + DEPTH, :],
                        in0=ct[poff:poff + DEPTH, :],
                        scalar1=vb_sb[poff:poff + DEPTH, mch:mch + 1],
                        scalar2=None, op0=ADD)

            # ---- Phase C: dense (row-parallel partial) ----
            with tc.tile_pool(name="osb", bufs=3) as osp, \
                 tc.tile_pool(name="psC", bufs=4, space="PSUM") as psC:
                for m in range(8):
                    os_t = osp.tile([P, S], F32, tag="os")
                    for n in range(2):
                        pd = psC.tile([P, 512], F32, tag="pd")
                        for kc in range(4):
                            nc.tensor.matmul(
                                pd[:],
                                ctxT[kc][:, m * P:(m + 1) * P],
                                dwT[kc][:, n * 512:(n + 1) * 512],
                                start=(kc == 0), stop=(kc == 3))
                        nc.vector.tensor_copy(
                            out=os_t[:, n * 512:(n + 1) * 512], in_=pd[:])
                    nc.sync.dma_start(out=outp[m * P:(m + 1) * P, :],
                                      in_=os_t[:])

    nc.compile()
    return nc


_PROGRAM_CACHE = {}


def get_program(mm_dt=MM_DT, es_dt=None):
    key = (str(mm_dt), str(es_dt))
    if key not in _PROGRAM_CACHE:
        _PROGRAM_CACHE[key] = build_program(mm_dt, es_dt)
    return _PROGRAM_CACHE[key]


def make_in_maps(v, k, q, mask, adjoin_matrix,
                 wq_w, wq_b, wk_w, wk_b, wv_w, wv_b, dense_w, dense_b):
    c = np.ascontiguousarray
    f32 = np.float32
    in_maps = []
    per_batch = {}
    for b in range(4):
        per_batch[b] = {
            "qT": c(np.asarray(q[b], f32).T),
            "kT": c(np.asarray(k[b], f32).T),
            "vT": c(np.asarray(v[b], f32).T),
            "adjT": c(np.asarray(adjoin_matrix[b, 0], f32).T)
            + np.float32(-1e9) * np.asarray(mask[b, 0, 0], f32)[:, None],
        }
    for cid in range(8):
        b, g = cid // 2, cid % 2
        gs = slice(g * W, (g + 1) * W)
        m = dict(per_batch[b])
        m["wqT"] = c(np.asarray(wq_w, f32)[gs].T) * f32(0.125)
        m["wkT"] = c(np.asarray(wk_w, f32)[gs].T)
        m["wvT"] = c(np.asarray(wv_w, f32)[gs].T)
        m["dwT"] = c(np.asarray(dense_w, f32)[:, gs].T)
        m["qb"] = c((np.asarray(wq_b, f32)[gs] * f32(0.125)).reshape(4, P).T)
        m["kb"] = c(np.asarray(wk_b, f32)[gs].reshape(4, P).T)
        m["vb"] = c(np.asarray(wv_b, f32)[gs].reshape(4, P).T)
        in_maps.append(m)
    return in_maps


def assemble_outputs(results, dense_b):
    out = np.empty((4, S, D), np.float32)
    attn = np.empty((4, H, S, S), np.float32)
    for cid in range(8):
        b, g = cid // 2, cid % 2
        au = results[cid]["attn_un"]          # [HC, sk, sq]
        z = au.sum(axis=1)                    # [HC, sq]
        attn[b, g * HC:(g + 1) * HC] = (au / z[:, None, :]).transpose(0, 2, 1)
    db = np.asarray(dense_b, np.float32)
    for b in range(4):
        out[b] = results[2 * b]["outp"] + results[2 * b + 1]["outp"] + db
    return out, attn


def run_cores(inputs, mm_dt=MM_DT, es_dt=None, trace=False, **run_kwargs):
    nc = get_program(mm_dt, es_dt)
    in_maps = make_in_maps(**inputs)
    res = run_bass_kernel_spmd(nc, in_maps, core_ids=list(range(8)),
                               trace=trace, **run_kwargs)
    return res


def kernel(**inputs):
    res = run_cores(inputs)
    return assemble_outputs(res.results, inputs["dense_b"])
